# revision 82
# baseline (speedup 1.0000x reference)
"""Trainium2 Bass kernel for nn_DilatedContextAttentionModule (B=8, C=256, 64x64).

Reference, per batch element (N = 64*64 = 4096):
    g   = G xj + g_b 1^T;  th = T xi + t_b 1^T;  phi = P xj + p_b 1^T
    f   = th^T phi / N                      (N x N, linear -- NO softmax)
    y[c,n] = sum_m f[n,m] g[c,m]
    z   = W y + W_b 1^T + xi
    out = BatchNorm2d(z)                    (training-mode batch stats)

Algebraic collapse (associativity; exact because f is linear):
    z  = (E' + I) xi + d 1^T,   E' = (1/N) W S T,  S = g phi^T
    with S = (G Xj)(P Xj)^T + u p_b^T + g_b v^T,
         u = G sxj + N g_b,  v = P sxj,  sxj = Xj 1.
Gram-form, with V := S^T (W/N)^T never materialized on the critical path:
    E'^T = (P^T T/64) [Gj (G^T W^T/64)] + (T^T p_b) u'^T + (T^T v) (W'g_b)^T
    with Gj = Xj Xj^T, u' = W'(G sxj + N g_b), v = P sxj.
All weight-by-weight products (G^T W^T, P^T T, ...) are precomputed on
the HOST, so the device never runs the 536 MMAC "conv": it computes the
Gram matrix Gj (268 MMAC via PE fp16 transposes + 32 accumulation
matmuls) plus a handful of 256x256 matmuls. 1/N is split as
(1/64)*(1/64) across the two host factors to stay inside fp16 normal
range. Each transposed tile carries an appended ones column, so the Gram
moving dim is 257 and its last column delivers sxj = Xj @ 1 for free.

Device pipeline (one batch element per core, 8 cores):
  phase G  transpose xj chunks with the PE (fp16 transpose = 53ns/tile),
           accumulate the augmented Gram in PSUM, software-pipelined two
           chunks deep.
  phase 2  t1 = Gj AG64;  E'^T = TP64 t1 + two rank-1 corrections + I;
           the d column is rebuilt from t1 during the phase-3 j=0 window
           (only needed for the BN stats correction and the tail affine).
  phase 3  z0 tiles [128, 512] = E_aug^T.T @ xi (d folded into the BN
           affine, NOT added here); DVE bn_stats per tile.
  BN       per-channel (mean, mean-of-squares)/8 for both chunks in ONE
           AllGather (out [8, 128, 4]) + local 3-add reduction; an
           AllGather has no reduce multiplier on the fabric and one
           launch overhead instead of the baseline's two AllReduces.
  stores   normalize (a*z0 + nb) in 512-col pieces, DVE/ACT split, each
           piece DMA'd as soon as it is ready.

Compute dtype fp16 (PE streams fp16 at 1 cycle/row like f32r; 10
mantissa bits keep the end-to-end rms relative error ~4e-4 vs the fp32
reference). Cost-model timeline: 68.7us (was 119.9us at baseline).
"""

import numpy as np

import concourse.bass as bass
import concourse.bacc as bacc
import concourse.tile as tile
from concourse import mybir
from concourse import bass_utils

B = 8
C = 256
N = 4096          # 64 * 64
NCORES = 8
NCH = 2           # channel chunks of 128
NT = 32           # n chunks of 128 (phase G)
NZ = 8            # n tiles of 512 (phase 3)
NQJ = 8           # xj DMA pieces (512 cols each)
NQI = 4           # xi DMA pieces (1024 cols each)
F32 = mybir.dt.float32
FP16 = mybir.dt.float16
BN_EPS = 1e-5

MM_DT = FP16


def build_kernel(nc, skip_cc: bool = False) -> None:
    f32 = F32
    xi_d = nc.dram_tensor("xi", [C, N], MM_DT, kind="ExternalInput").ap()
    xj_d = nc.dram_tensor("xj", [C, N], MM_DT, kind="ExternalInput").ap()
    # [128, 128]: identity (transpose permutation operand)
    idt_d = nc.dram_tensor("idt", [128, 128], MM_DT, kind="ExternalInput").ap()
    # [128, 2, 256]: (G^T W^T)/64, chunked on the contraction index
    ag64_d = nc.dram_tensor("ag64", [128, NCH, C], MM_DT, kind="ExternalInput").ap()
    # [128, 2, 256]: P^T/64, chunked on the contraction index
    pt64_d = nc.dram_tensor("pt64", [128, NCH, C], MM_DT, kind="ExternalInput").ap()
    # [128, 2, 256]: (P^T T)/64, chunked on the contraction index
    tp64_d = nc.dram_tensor("tp64", [128, NCH, C], MM_DT, kind="ExternalInput").ap()
    # [128, 2, 256]: theta_w rows, chunked (for tvrow)
    wtw_d = nc.dram_tensor("wtw", [128, NCH, C], MM_DT, kind="ExternalInput").ap()
    # [128, 2]: theta_b column, chunked
    wtb_d = nc.dram_tensor("wtb", [128, NCH], MM_DT, kind="ExternalInput").ap()
    # [128, 2]: (P t_b)/64 column, chunked
    qtb_d = nc.dram_tensor("qtb", [128, NCH], MM_DT, kind="ExternalInput").ap()
    # [1, 3*256+64]: rows [W g_b | (W g_b)/64 | T^T p_b | consts]
    aux_d = nc.dram_tensor("aux", [1, 3 * C + 64], MM_DT, kind="ExternalInput").ap()
    # [128, 2, 2]: (gamma, beta) per channel, chunked
    gbe_d = nc.dram_tensor("gbe", [128, NCH, 2], f32, kind="ExternalInput").ap()
    # [128, 2]: W_b column, chunked
    wbc_d = nc.dram_tensor("wbc", [128, NCH], f32, kind="ExternalInput").ap()
    # [128, 2, 256]: identity matrix chunks (for E'^T + I)
    idn_d = nc.dram_tensor("idn", [128, NCH, C], MM_DT, kind="ExternalInput").ap()
    out_d = nc.dram_tensor("out", [C, N], f32, kind="ExternalOutput").ap()

    with tile.TileContext(nc) as tc:
        _body(tc, xi_d, xj_d, idt_d, ag64_d, pt64_d, tp64_d, wtw_d, wtb_d,
              qtb_d, aux_d, gbe_d, idn_d, wbc_d, out_d, skip_cc=skip_cc)


def _body(tc, xi_d, xj_d, idt_d, ag64_d, pt64_d, tp64_d, wtw_d, wtb_d,
          qtb_d, aux_d, gbe_d, idn_d, wbc_d, out_d, skip_cc: bool = False):
    nc = tc.nc
    f32 = F32
    import contextlib

    with contextlib.ExitStack() as ctx:
        constp = ctx.enter_context(tc.tile_pool(name="const", bufs=1))
        datap = ctx.enter_context(tc.tile_pool(name="data", bufs=1))
        workp = ctx.enter_context(tc.tile_pool(name="work", bufs=4))
        rowsp = ctx.enter_context(tc.tile_pool(name="rows", bufs=2))
        psbig = ctx.enter_context(tc.tile_pool(name="ps_big", bufs=4, space="PSUM"))
        psacc = ctx.enter_context(tc.tile_pool(name="ps_acc", bufs=2, space="PSUM"))
        pssml = ctx.enter_context(tc.tile_pool(name="ps_sml", bufs=2, space="PSUM"))
        dramp = ctx.enter_context(tc.tile_pool(name="dram", bufs=2, space="DRAM"))

        mdt = MM_DT
        # ---- loads: transpose identity first, then the data streams.
        # Sync queue (HWDGE): no Pool desc-gen serialization.
        idt = constp.tile([128, 128], mdt, tag="idt")
        nc.sync.dma_start(out=idt, in_=idt_d)
        JW = N // NQJ
        xj_h = []
        for h in range(NQJ):
            t = datap.tile([128, NCH, JW], mdt, tag=f"xjh{h}", name=f"xj_h{h}")
            nc.sync.dma_start(
                out=t,
                in_=xj_d.rearrange("(k p) n -> p k n", p=128)[:, :, h * JW:(h + 1) * JW],
            )
            xj_h.append(t)
        ag64 = constp.tile([128, NCH, C], mdt, tag="ag64")
        nc.sync.dma_start(out=ag64, in_=ag64_d)
        pt64 = constp.tile([128, NCH, C], mdt, tag="pt64")
        nc.sync.dma_start(out=pt64, in_=pt64_d)
        tp64 = constp.tile([128, NCH, C], mdt, tag="tp64")
        nc.sync.dma_start(out=tp64, in_=tp64_d)
        w_tw = constp.tile([128, NCH, C], mdt, tag="w_tw")
        nc.sync.dma_start(out=w_tw, in_=wtw_d)
        w_tb = constp.tile([128, NCH], mdt, tag="w_tb")
        nc.sync.dma_start(out=w_tb, in_=wtb_d)
        qtb = constp.tile([128, NCH], mdt, tag="qtb")
        nc.sync.dma_start(out=qtb, in_=qtb_d)
        aux = constp.tile([1, 3 * C + 64], mdt, tag="aux")
        nc.sync.dma_start(out=aux, in_=aux_d)
        idn = constp.tile([128, NCH, C], mdt, tag="idn")
        nc.sync.dma_start(out=idn, in_=idn_d)
        # ---- phase-2 weights and small constants: also on the sync queue
        # (after the big loads) so the Pool engine has NO desc-gen work and
        # is free to run the sxj reduction
        IW = N // NQI
        xi_h = []
        for h in range(NQI):
            t = datap.tile([128, NCH, IW], mdt, tag=f"xih{h}", name=f"xi_h{h}")
            nc.sync.dma_start(
                out=t,
                in_=xi_d.rearrange("(k p) n -> p k n", p=128)[:, :, h * IW:(h + 1) * IW],
            )
            xi_h.append(t)
        gbe = constp.tile([128, NCH, 2], f32, tag="gbe")
        nc.sync.dma_start(out=gbe, in_=gbe_d)
        wbc = constp.tile([128, NCH], f32, tag="wbc")
        nc.sync.dma_start(out=wbc, in_=wbc_d)
        wgbrow = aux[:, 0:C]
        wgb64row = aux[:, C:2 * C]
        tpbrow = aux[:, 2 * C:3 * C]    # (T^T p_b)^T
        c64 = aux[:, 3 * C:3 * C + 1]   # 1/64
        cpt = aux[:, 3 * C + 1:3 * C + 2]   # p_b . t_b
        eps = constp.tile([128, 1], f32, tag="eps")
        nc.vector.memset(eps, BN_EPS)
        # preload both activation-function tables (Identity for the copies,
        # Sqrt for the BN tail) while the ACT engine is still idle
        warm = rowsp.tile([128, 1], f32, tag="warm")
        nc.scalar.activation(
            out=warm, in_=eps, func=mybir.ActivationFunctionType.Identity,
            bias=eps, scale=1.0,
        )
        nc.scalar.activation(
            out=warm, in_=eps, func=mybir.ActivationFunctionType.Sqrt,
            bias=eps, scale=1.0,
        )
        # warm-up matmuls: keep the PE busy from t~0.5us so the p-state ramp
        # (full clock only after 3us of continuous execution) completes
        # before the real work arrives
        warm64 = constp.tile([128, 64], f32, tag="warm64")
        nc.vector.memset(warm64, 0.0)
        warm_ps = pssml.tile([1, 64], f32, tag="sml", name="warm_ps")
        for _ in range(14):
            nc.tensor.matmul(warm_ps, eps, warm64, start=True, stop=True)

        def xi_sl(k, tix):
            # phase-3 tile tix of 512 columns, channel-chunk k
            h, off = divmod(tix * 512, IW)
            return xi_h[h][:, k, off:off + 512]

        def xj_sl(k, i):
            # chunk i of 128 columns, channel-chunk k
            h, off = divmod(i * 128, JW)
            return xj_h[h][:, k, off:off + 128]

        # ---- phase G: Gj = Xj Xj^T via PE transposes -----------------
        # software-pipelined two chunks deep: transposes for chunk i+2 are
        # emitted BEFORE the Gram accumulation of chunk i so the in-order PE
        # queue never stalls on the PSUM->SBUF copy of the transposed tile.
        # Each transposed tile carries an extra ones column, so the Gram's
        # moving dim is 257 and its last column delivers sxj = Xj @ 1 for
        # free -- no separate rowsum machinery at all.
        CA = C + 1
        Gj_ps = [psacc.tile([128, CA], f32, tag="acc", name=f"Gj_ps{m}")
                 for m in range(NCH)]
        tis = []

        def trans(i):
            # transpose PSUM staging reuses the [128,512]f32 "big" bank pool
            # via a bitcast view, so one pool serves both phase G and phase 3
            t_raw = psbig.tile([128, 512], f32, tag="big", name=f"tps{i}")
            t_ps = t_raw.bitcast(MM_DT)[:, 0:C]
            for k in range(NCH):
                nc.tensor.transpose(
                    t_ps[:, k * 128:(k + 1) * 128], xj_sl(k, i), idt)
            ti = workp.tile([128, CA], mdt, tag="ti", name=f"ti{i}")
            nc.vector.tensor_copy(ti[:, 0:C], t_ps)
            nc.vector.memset(ti[:, C:CA], 1.0)
            tis.append(ti)

        r1_ps = pssml.tile([1, C], f32, tag="sml", name="r1_ps")
        uprow = rowsp.tile([1, C], mdt, tag="uprow")

        def sxj_col(k):
            return Gj_sb[k][:, C:CA]

        def emit_rank1_rows():
            # r1 = ((W G)/64 sxj)^T;  u'row = r1/64 + (W g_b)^T
            for k in range(NCH):
                nc.tensor.matmul(
                    r1_ps, sxj_col(k), ag64[:, k, :],
                    start=(k == 0), stop=(k == NCH - 1),
                )
            nc.vector.scalar_tensor_tensor(
                out=uprow, in0=r1_ps, scalar=c64, in1=wgbrow,
                op0=mybir.AluOpType.mult, op1=mybir.AluOpType.add,
            )

        trans(0)
        trans(1)
        for i in range(NT):
            if i + 2 < NT:
                trans(i + 2)
            ti = tis[i]
            for m in range(NCH):
                nc.tensor.matmul(
                    Gj_ps[m],
                    ti[:, m * 128:(m + 1) * 128],
                    ti,
                    start=(i == 0), stop=(i == NT - 1),
                )
        Gj_sb = []
        for m in range(NCH):
            t = workp.tile([128, CA], mdt, tag=f"Gj{m}")
            if m == 0:
                nc.vector.tensor_copy(t, Gj_ps[m])
            else:
                nc.scalar.copy(t, Gj_ps[m])
            Gj_sb.append(t)

        # ---- phase 2: t1 = Gj AG64; E'^T = TP64 t1 + rank-1s --------
        # V never materializes on the critical chain: E'^T = T^T V is
        # computed directly via the host-folded TP64 = (P^T T)/64, and the
        # d column is rebuilt from t1 during the phase-3 j=0 window.
        # small rank-1 ingredients first (need only the sxj columns of Gja)
        t1_ps = []
        for m in range(NCH):
            t_ps = psacc.tile([128, C], f32, tag="acc", name=f"t1_ps{m}")
            msl = slice(m * 128, (m + 1) * 128)
            for k in range(NCH):
                nc.tensor.matmul(
                    t_ps, Gj_sb[k][:, msl], ag64[:, k, :],
                    start=(k == 0), stop=(k == NCH - 1),
                )
            t1_ps.append(t_ps)
        # rank-1 row matmuls overlap the t1 PSUM->SBUF copies below
        emit_rank1_rows()
        v64c_ps = pssml.tile([128, NCH], f32, tag="sml", name="v64c_ps")
        for c2 in range(NCH):
            for k in range(NCH):
                nc.tensor.matmul(
                    v64c_ps[:, c2:c2 + 1],
                    pt64[:, k, c2 * 128:(c2 + 1) * 128],
                    sxj_col(k),
                    start=(k == 0), stop=(k == NCH - 1),
                )
        v64col = rowsp.tile([128, NCH], mdt, tag="v64col")
        nc.vector.tensor_copy(v64col, v64c_ps)
        # tvrow = (T^T v64)^T ; svt = v64 . t_b  (for the d column later)
        tv_ps = pssml.tile([1, C + 1], f32, tag="sml", name="tv_ps")
        for c2 in range(NCH):
            nc.tensor.matmul(
                tv_ps[:, 0:C], v64col[:, c2:c2 + 1], w_tw[:, c2, :],
                start=(c2 == 0), stop=(c2 == NCH - 1),
            )
        for c2 in range(NCH):
            nc.tensor.matmul(
                tv_ps[:, C:C + 1], v64col[:, c2:c2 + 1], w_tb[:, c2:c2 + 1],
                start=(c2 == 0), stop=(c2 == NCH - 1),
            )
        tvrow = rowsp.tile([1, C + 1], mdt, tag="tvrow")
        nc.vector.tensor_copy(tvrow, tv_ps)
        t1_sb = []
        for m in range(NCH):
            t = workp.tile([128, C], mdt, tag=f"t1{m}")
            if m == 0:
                nc.vector.tensor_copy(t, t1_ps[m])
            else:
                nc.scalar.copy(t, t1_ps[m])
            t1_sb.append(t)
        ET_sb = []
        for m in range(NCH):
            e_ps = psacc.tile([128, C], f32, tag="acc")
            msl = slice(m * 128, (m + 1) * 128)
            for k in range(NCH):
                nc.tensor.matmul(
                    e_ps, tp64[:, k, msl], t1_sb[k],
                    start=(k == 0), stop=False,
                )
            # E'^T += (T^T p_b) (u')^T + (T^T v64) (W g_b / 64)^T
            nc.tensor.matmul(
                e_ps, tpbrow[:, msl], uprow, start=False, stop=False)
            nc.tensor.matmul(
                e_ps, tvrow[:, msl], wgb64row, start=False, stop=True)
            t = workp.tile([128, C], mdt, tag=f"ET{m}")
            nc.vector.tensor_add(t, e_ps, idn[:, m, :])
            ET_sb.append(t)

        # ---- phase 3: z0 = (E'+I)^T.T @ xi; BN stats fused ----------
        z_t = datap.tile([128, NCH, N], f32, tag="z")
        spack = rowsp.tile([128, 4], f32, tag="spack")
        cc_in = dramp.tile([128, 4], f32, tag="cc_in", name="cc_in")
        cc_out = dramp.tile([NCORES, 128, 4], f32, tag="cc_out", name="cc_out")
        dcol_ps = pssml.tile([128, NCH], f32, tag="sml")
        dcol = rowsp.tile([128, NCH], f32, tag="dcol")
        for j in range(NCH):
            jsl = slice(j * 128, (j + 1) * 128)
            stats = workp.tile([128, NZ, 6], f32, tag="bnst", name=f"stats{j}")
            for tix in range(NZ):
                tsl = slice(tix * 512, (tix + 1) * 512)
                z_ps = psbig.tile([128, 512], f32, tag="big")
                for k in range(NCH):
                    nc.tensor.matmul(
                        z_ps, ET_sb[k][:, jsl], xi_sl(k, tix),
                        start=(k == 0), stop=(k == NCH - 1),
                    )
                nc.scalar.copy(z_t[:, j, tsl], z_ps)
                nc.vector.bn_stats(out=stats[:, tix, :], in_=z_t[:, j, tsl])
            if j == 0:
                # d column = V^T t_b + W_b rebuilt from t1 (V is never
                # materialized), off the phase-3 critical entry:
                # d = t1^T (P^T t_b)/64 + uprow^T (p_b.t_b)
                #     + wgb64^T (v64.t_b) + W_b
                for jj in range(NCH):
                    jjsl = slice(jj * 128, (jj + 1) * 128)
                    for k in range(NCH):
                        nc.tensor.matmul(
                            dcol_ps[:, jj:jj + 1],
                            t1_sb[k][:, jjsl],
                            qtb[:, k:k + 1],
                            start=(k == 0), stop=False,
                        )
                    nc.tensor.matmul(
                        dcol_ps[:, jj:jj + 1], uprow[:, jjsl], cpt,
                        start=False, stop=False,
                    )
                    nc.tensor.matmul(
                        dcol_ps[:, jj:jj + 1], wgb64row[:, jjsl],
                        tvrow[:, C:C + 1],
                        start=False, stop=True,
                    )
                nc.vector.tensor_add(dcol, dcol_ps, wbc)
            mv = rowsp.tile([128, 2], f32, tag="mv")
            nc.vector.bn_aggr(out=mv, in_=stats)
            # true mean = mean(z0) + d;  spack = (mean/8, (mean^2+var)/8)
            mt = rowsp.tile([128, 1], f32, tag="mt")
            nc.vector.tensor_add(mt, mv[:, 0:1], dcol[:, j:j + 1])
            nc.vector.tensor_scalar_mul(
                spack[:, 2 * j:2 * j + 1], mt, 1.0 / NCORES)
            nc.vector.scalar_tensor_tensor(
                out=spack[:, 2 * j + 1:2 * j + 2], in0=mt,
                scalar=mt, in1=mv[:, 1:2],
                op0=mybir.AluOpType.mult, op1=mybir.AluOpType.add,
            )
            nc.vector.tensor_scalar_mul(
                spack[:, 2 * j + 1:2 * j + 2],
                spack[:, 2 * j + 1:2 * j + 2], 1.0 / NCORES)
            # stage this chunk's stats to DRAM immediately: chunk 0's DMA
            # overlaps chunk 1's compute, only chunk 1's is exposed
            nc.sync.dma_start(
                out=cc_in[:, 2 * j:2 * j + 2], in_=spack[:, 2 * j:2 * j + 2])

        # ---- ONE AllGather for both chunks' stats; local 8-way sum --
        if skip_cc:
            nc.sync.dma_start(out=cc_out[0, :, :], in_=cc_in)
        else:
            nc.gpsimd.collective_compute(
                "AllGather",
                mybir.AluOpType.bypass,
                replica_groups=[list(range(NCORES))],
                ins=[cc_in.opt()],
                outs=[cc_out.opt()],
            )
        sall = rowsp.tile([128, NCORES, 4], f32, tag="sall")
        nc.sync.dma_start(
            out=sall, in_=cc_out.rearrange("r p s -> p r s"))
        s4 = rowsp.tile([128, 4, 4], f32, tag="s4")
        nc.vector.tensor_add(s4, sall[:, 0:4, :], sall[:, 4:8, :])
        s2 = rowsp.tile([128, 2, 4], f32, tag="s2")
        nc.vector.tensor_add(s2, s4[:, 0:2, :], s4[:, 2:4, :])
        ssum = rowsp.tile([128, 4], f32, tag="ssum")
        nc.vector.tensor_add(ssum, s2[:, 0, :], s2[:, 1, :])

        # ---- normalize + affine + store -----------------------------
        # affine vectors for BOTH chunks at once on [128, 2] strided views
        mcols = ssum[:, 0:4:2]
        qcols = ssum[:, 1:4:2]
        # negvar = m^2 - q  (sqrt uses scale=-1 to flip the sign)
        nv2 = rowsp.tile([128, 2], f32, tag="nv2")
        nc.vector.tensor_mul(nv2, mcols, mcols)
        nc.vector.tensor_sub(nv2, nv2, qcols)
        # nm0 = d - m = -(global mean of z0); independent of the sqrt chain
        nm02 = rowsp.tile([128, 2], f32, tag="nm02")
        nc.vector.tensor_sub(nm02, dcol, mcols)
        # sc = sqrt(var + eps);  a = gamma / sc
        sc2 = rowsp.tile([128, 2], f32, tag="sc2")
        nc.scalar.activation(
            out=sc2, in_=nv2, func=mybir.ActivationFunctionType.Sqrt,
            bias=eps, scale=-1.0,
        )
        nc.vector.reciprocal(out=sc2, in_=sc2)
        ac2 = rowsp.tile([128, 2], f32, tag="ac2")
        nc.vector.tensor_mul(ac2, sc2, gbe[:, :, 0])
        # nb = beta - a*(m - d):  out = a*z0 + nb
        nb2 = rowsp.tile([128, 2], f32, tag="nb2")
        nc.vector.tensor_mul(nb2, nm02, ac2)
        nc.vector.tensor_add(nb2, nb2, gbe[:, :, 1])
        # normalize in 512-col pieces, DVE and ACT alternating, store each
        # piece as soon as it is ready so the output DMA pipeline starts
        # right after the collective
        PW = N // 8
        for j in range(NCH):
            acol = ac2[:, j:j + 1]
            nbcol = nb2[:, j:j + 1]
            for p in range(8):
                psl = slice(p * PW, (p + 1) * PW)
                if p % 2 == 0:
                    nc.vector.tensor_scalar(
                        out=z_t[:, j, psl], in0=z_t[:, j, psl],
                        scalar1=acol, scalar2=nbcol,
                        op0=mybir.AluOpType.mult, op1=mybir.AluOpType.add,
                    )
                else:
                    nc.scalar.activation(
                        out=z_t[:, j, psl], in_=z_t[:, j, psl],
                        func=mybir.ActivationFunctionType.Identity,
                        bias=nbcol, scale=acol,
                    )
                nc.sync.dma_start(
                    out=out_d[j * 128:(j + 1) * 128, psl], in_=z_t[:, j, psl])


_NC_CACHE: dict = {}


def _get_nc():
    if "nc" not in _NC_CACHE:
        nc = bacc.Bacc(
            "TRN2",
            target_bir_lowering=False,
            debug=False,
            enable_asserts=True,
            num_devices=NCORES,
        )
        build_kernel(nc)
        nc.compile()
        _NC_CACHE["nc"] = nc
    return _NC_CACHE["nc"]


def _make_in_maps(inputs: dict) -> list[dict]:
    f16 = np.float16
    xi = np.ascontiguousarray(
        np.asarray(inputs["xi"], np.float32).reshape(B, C, N).astype(f16))
    xj = np.ascontiguousarray(
        np.asarray(inputs["xj"], np.float32).reshape(B, C, N).astype(f16))
    g_w = np.asarray(inputs["g_w"], np.float32)
    g_b = np.asarray(inputs["g_b"], np.float32)
    t_w = np.asarray(inputs["theta_w"], np.float32)
    t_b = np.asarray(inputs["theta_b"], np.float32)
    p_w = np.asarray(inputs["phi_w"], np.float32)
    p_b = np.asarray(inputs["phi_b"], np.float32)
    W_w = np.asarray(inputs["W_w"], np.float32)
    W_b = np.asarray(inputs["W_b"], np.float32)
    gam = np.asarray(inputs["bn_gamma"], np.float32)
    bet = np.asarray(inputs["bn_beta"], np.float32)

    def chunked(a):  # [256, F] -> [128, 2, F]
        return np.ascontiguousarray(a.reshape(2, 128, -1).transpose(1, 0, 2))

    idt = np.eye(128, dtype=np.float32).astype(f16)                # [128,128]
    ag64 = chunked(g_w.T @ W_w.T / 64.0).astype(f16)               # [128,2,256]
    pt64 = chunked(p_w.T / 64.0).astype(f16)                       # [128,2,256]
    tp64 = chunked(p_w.T @ t_w / 64.0).astype(f16)                 # [128,2,256]
    wtw = chunked(t_w).astype(f16)                                 # [128,2,256]
    wtb = np.ascontiguousarray(t_b.reshape(2, 128).T).astype(f16)  # [128,2]
    qtb = np.ascontiguousarray(
        (p_w.T @ t_b / 64.0).reshape(2, 128).T).astype(f16)        # [128,2]
    wgb = W_w @ g_b                                                # [256]
    consts = np.zeros(64, np.float32)
    consts[0] = 1.0 / 64.0
    consts[1] = float(p_b @ t_b)
    aux = np.concatenate([wgb, wgb / 64.0, t_w.T @ p_b, consts])[None, :]
    aux = np.ascontiguousarray(aux.astype(f16))                    # [1,832]
    gbe = chunked(np.stack([gam, bet], axis=1))                    # [128,2,2]
    idn = chunked(np.eye(C, dtype=np.float32)).astype(f16)         # [128,2,256]
    wbc = np.ascontiguousarray(W_b.reshape(2, 128).T)              # [128,2]

    in_maps = []
    for b in range(B):
        in_maps.append({
            "xi": xi[b], "xj": xj[b],
            "idt": idt, "ag64": ag64, "pt64": pt64, "tp64": tp64,
            "wtw": wtw, "wtb": wtb, "qtb": qtb,
            "aux": aux, "gbe": gbe, "idn": idn, "wbc": wbc,
        })
    return in_maps


def kernel(**inputs) -> np.ndarray:
    nc = _get_nc()
    in_maps = _make_in_maps(inputs)
    last_err = None
    for attempt in range(3):
        try:
            res = bass_utils.run_bass_kernel_spmd(
                nc, in_maps, core_ids=list(range(NCORES)),
            )
            break
        except Exception as e:  # transient device wedge: back off and retry
            last_err = e
            import time as _time
            _time.sleep(4.0 * (attempt + 1))
            try:
                import jax
                import jax.extend.backend as _jeb
                jax.clear_caches()
                # tear down the PJRT client: a fresh axon connection lets the
                # terminal reset a wedged exec unit
                _jeb.clear_backends()
            except Exception:
                pass
    else:
        raise last_err
    out = np.stack([res.results[c]["out"] for c in range(NCORES)])
    return np.ascontiguousarray(out.reshape(B, C, 64, 64).astype(np.float32))


if __name__ == "__main__":
    rng = np.random.default_rng(0)
    fake = {
        "xi": rng.standard_normal((B, C, 64, 64), np.float32),
        "xj": rng.standard_normal((B, C, 64, 64), np.float32),
        "g_w": rng.standard_normal((C, C), np.float32) / 16,
        "g_b": rng.standard_normal((C,), np.float32) / 16,
        "theta_w": rng.standard_normal((C, C), np.float32) / 16,
        "theta_b": rng.standard_normal((C,), np.float32) / 16,
        "phi_w": rng.standard_normal((C, C), np.float32) / 16,
        "phi_b": rng.standard_normal((C,), np.float32) / 16,
        "W_w": rng.standard_normal((C, C), np.float32) / 16,
        "W_b": rng.standard_normal((C,), np.float32) / 16,
        "bn_gamma": np.ones((C,), np.float32),
        "bn_beta": np.zeros((C,), np.float32),
    }
    out = kernel(**fake)
    print("out", out.shape, out.dtype, float(np.abs(out).mean()))
